# revision 8
# baseline (speedup 1.0000x reference)
"""Low-rank (CPD) 3D conv kernel for Trainium2, SPMD across 8 NeuronCores.

Math (per reference):
  y[r,h,w,d]  = sum_c U_c_in[c,r] * x[c,h,w,d]
  y           = conv_h(conv_w(conv_d-separable 3-tap, per-rank taps U_k*))
  out[c,...]  = sum_r U_c_out[r,c] * z[r,...] + bias[c]

Distribution: data-parallel split of H (64) into 8 slabs of 8 planes; each
core reads its slab plus one halo plane on each side (zero at global edges)
and computes its output slab independently. No collectives.

Per-core pipeline (streamed over the 8 output planes):
  - mm1 with conv_h folded: 3 weight matrices W_k = U_c_in * U_kh[k] (host
    precomputed, bf16); PSUM accumulation over 2 c-tiles x 3 h-taps.
  - PSUM drain on ScalarE, casting to bf16 and de-interleaving d into
    (even,odd) halves per w-line so the d-shifts below stay 4B-aligned.
  - conv_w on VectorE: per-partition scale (tensor_scalar) + 2 fused
    scale-add passes (scalar_tensor_tensor) with +-1 w-line shifts.
  - conv_d on VectorE: same, operating across the even/odd halves.
  - mm2: lhsT = U_c_out (bf16), accumulate 2 r-tiles.
  - PSUM drain on ScalarE with per-partition bias add, re-interleaving d,
    f32 output.
"""

import numpy as np
import ml_dtypes

BF16 = ml_dtypes.bfloat16

# Problem constants (hardcoded per contest contract)
C = 256   # input channels
R = 256   # rank
CO = 256  # output channels
S = 64    # spatial extent (cube)
NCORES = 8
HP = S // NCORES          # output planes per core (8)
HS = HP + 2               # slab planes incl. halo (10)
PLANE = S * S             # 4096 elements per (w,d) plane
NCH = PLANE // 512        # 512-column matmul chunks per plane (8)

_cache = {}


def _build_program(hp=HP, wl=S):
    """Build and compile the per-core Bass program (identical on all cores).

    hp: output planes per core; wl: w-lines per plane (64 in production).
    """
    import concourse.bass as bass
    import concourse.mybir as mybir
    import concourse.tile as tile
    from concourse import bacc

    HS, PLANE, NCH = hp + 2, wl * 64, (wl * 64) // 512
    HP_ = hp

    fp32 = mybir.dt.float32
    bf16 = mybir.dt.bfloat16

    nc = bacc.Bacc("TRN2", target_bir_lowering=False, debug=False,
                   num_devices=NCORES)

    # DRAM tensors (names are the in_map keys)
    x_d = nc.dram_tensor("xs", [2, 128, HS, PLANE], bf16, kind="ExternalInput").ap()
    wkh_d = nc.dram_tensor("wkh", [3, 2, 2, 128, 128], bf16, kind="ExternalInput").ap()
    uco_d = nc.dram_tensor("uco", [2, 2, 128, 128], bf16, kind="ExternalInput").ap()
    ukw_d = nc.dram_tensor("ukw", [2, 128, 3], fp32, kind="ExternalInput").ap()
    ukd_d = nc.dram_tensor("ukd", [2, 128, 3], fp32, kind="ExternalInput").ap()
    bias_d = nc.dram_tensor("bias_t", [2, 128, 1], fp32, kind="ExternalInput").ap()
    out_d = nc.dram_tensor("out", [2, 128, HP_, PLANE], bf16, kind="ExternalOutput").ap()

    mult = mybir.AluOpType.mult
    add = mybir.AluOpType.add
    ident = mybir.ActivationFunctionType.Identity

    with tile.TileContext(nc) as tc:
        consts = tc.alloc_tile_pool(name="consts", bufs=1)
        xpool = tc.alloc_tile_pool(name="x", bufs=8)
        ypool = tc.alloc_tile_pool(name="y", bufs=4)
        tpool = tc.alloc_tile_pool(name="tmp", bufs=6)
        zpool = tc.alloc_tile_pool(name="z", bufs=2)
        zdpool = tc.alloc_tile_pool(name="zd", bufs=3)
        opool = tc.alloc_tile_pool(name="osb", bufs=2)
        ps1 = tc.alloc_tile_pool(name="ps1", bufs=2, space="PSUM")
        ps2 = tc.alloc_tile_pool(name="ps2", bufs=2, space="PSUM")

        # ---- constants ----
        wkh = [[[consts.tile([128, 128], bf16, name=f"wkh{k}{ct}{rt}", tag=f"wkh{k}{ct}{rt}")
                 for rt in range(2)] for ct in range(2)] for k in range(3)]
        for k in range(3):
            for ct in range(2):
                for rt in range(2):
                    nc.sync.dma_start(out=wkh[k][ct][rt], in_=wkh_d[k, ct, rt])
        uco = [[consts.tile([128, 128], bf16, name=f"uco{rt}{co}", tag=f"uco{rt}{co}")
                for co in range(2)] for rt in range(2)]
        for rt in range(2):
            for co in range(2):
                nc.sync.dma_start(out=uco[rt][co], in_=uco_d[rt, co])
        ukw = [consts.tile([128, 3], fp32, name=f"ukw{rt}", tag=f"ukw{rt}") for rt in range(2)]
        ukd = [consts.tile([128, 3], fp32, name=f"ukd{rt}", tag=f"ukd{rt}") for rt in range(2)]
        bia = [consts.tile([128, 1], fp32, name=f"bias{co}", tag=f"bias{co}") for co in range(2)]
        for rt in range(2):
            nc.sync.dma_start(out=ukw[rt], in_=ukw_d[rt])
            nc.sync.dma_start(out=ukd[rt], in_=ukd_d[rt])
        for co in range(2):
            nc.sync.dma_start(out=bia[co], in_=bias_d[co])

        # ---- x plane streaming ----
        xt = {}

        def get_x(p, ct):
            if (p, ct) not in xt:
                t = xpool.tile([128, PLANE], bf16, name="xplane", tag="xplane")
                nc.sync.dma_start(out=t, in_=x_d[ct, :, p, :])
                xt[(p, ct)] = t
            return xt[(p, ct)]

        NQ = PLANE // 1024  # 1024-wide psum tiles per plane

        for h in range(HP_):
            y = []
            t0s = []
            for rt in range(2):
                # --- mm1 + conv_h fold (PSUM 1024-tiles, 512 matmul halves) ---
                ysb = ypool.tile([128, PLANE], bf16, name="ysb", tag="y")
                t0 = tpool.tile([128, PLANE], bf16, name="t0t", tag="tmp")
                for q in range(NQ):
                    pt = ps1.tile([128, 1024], fp32, name="pt1", tag="ps1")
                    for half in range(2):
                        first = True
                        for k in range(3):
                            for ct in range(2):
                                nc.tensor.matmul(
                                    pt[:, half * 512:(half + 1) * 512],
                                    wkh[k][ct][rt],
                                    get_x(h + k, ct)[:, q * 1024 + half * 512:
                                                     q * 1024 + (half + 1) * 512],
                                    start=first,
                                    stop=(k == 2 and ct == 1),
                                )
                                first = False
                    # drains: f32 PSUM -> bf16 SBUF, de-interleave d.
                    # plain y (ACT); U_kw[0]-scaled t0 from ACT for rt0 and
                    # from a DVE mul for rt1 (ACT/DVE load balance).
                    src = pt.rearrange("p (w j s) -> p w s j", j=32, s=2)
                    dst = ysb.rearrange("p (w s j) -> p w s j", s=2, j=32)[
                        :, q * 16:(q + 1) * 16]
                    nc.scalar.copy(dst, src)
                    if rt == 0:
                        dst0 = t0.rearrange("p (w s j) -> p w s j", s=2, j=32)[
                            :, q * 16:(q + 1) * 16]
                        nc.scalar.mul(dst0, src, ukw[rt][:, 0:1])
                if rt == 1:
                    nc.vector.tensor_scalar_mul(t0, ysb, ukw[rt][:, 0:1])
                y.append(ysb)
                t0s.append(t0)

            # --- conv_w (VectorE + tmp from ACT drains) ---
            z = []
            for rt in range(2):
                zt = zpool.tile([128, PLANE], bf16, name="zw", tag="z")
                # z = U1*y
                nc.vector.tensor_scalar_mul(zt, y[rt], ukw[rt][:, 1:2])
                zv = zt.rearrange("p (w q) -> p w q", q=64)
                t0v = t0s[rt].rearrange("p (w q) -> p w q", q=64)
                yv = y[rt].rearrange("p (w q) -> p w q", q=64)
                # z[w] += t0[w-1]
                nc.vector.tensor_tensor(zv[:, 1:, :], t0v[:, :-1, :], zv[:, 1:, :], add)
                # t2 = U2*y ; z[w] += t2[w+1]
                t2 = tpool.tile([128, PLANE], bf16, name="t2t", tag="tmp")
                nc.vector.tensor_scalar_mul(t2, y[rt], ukw[rt][:, 2:3])
                t2v = t2.rearrange("p (w q) -> p w q", q=64)
                nc.vector.tensor_tensor(zv[:, :-1, :], t2v[:, 1:, :], zv[:, :-1, :], add)
                z.append(zt)

            # --- conv_d (VectorE scales; even-chain on DVE, odd-chain on
            # GpSimd so the two 2-op add chains run in parallel) ---
            zd = []
            for rt in range(2):
                zt = zdpool.tile([128, PLANE], bf16, name="zdt", tag="zd")
                a0 = tpool.tile([128, PLANE], bf16, name="a0t", tag="tmp")
                a2 = tpool.tile([128, PLANE], bf16, name="a2t", tag="tmp")
                nc.vector.tensor_scalar_mul(zt, z[rt], ukd[rt][:, 1:2])
                nc.vector.tensor_scalar_mul(a0, z[rt], ukd[rt][:, 0:1])
                nc.vector.tensor_scalar_mul(a2, z[rt], ukd[rt][:, 2:3])
                zv = zt.rearrange("p (w s j) -> p w s j", s=2, j=32)
                a0v = a0.rearrange("p (w s j) -> p w s j", s=2, j=32)
                a2v = a2.rearrange("p (w s j) -> p w s j", s=2, j=32)
                # 4B-aligned adds on DVE (fast 2x mode); the two j+-1 "wrap"
                # adds have 2-byte-offset APs that cripple DVE, so they go to
                # GpSimd which is alignment-insensitive.
                nc.vector.tensor_tensor(zv[:, :, 0, :], a2v[:, :, 1, :], zv[:, :, 0, :], add)
                nc.vector.tensor_tensor(zv[:, :, 1, :], a0v[:, :, 0, :], zv[:, :, 1, :], add)
                nc.gpsimd.tensor_tensor(zv[:, :, 0, 1:], a0v[:, :, 1, :-1], zv[:, :, 0, 1:], add)
                nc.gpsimd.tensor_tensor(zv[:, :, 1, :-1], a2v[:, :, 0, 1:], zv[:, :, 1, :-1], add)
                zd.append(zt)

            # --- mm2 + bias drain (bf16 out; host upcasts) ---
            for co in range(2):
                osb = opool.tile([128, PLANE], bf16, name="osb", tag="osb")
                for q in range(NQ):
                    pt = ps2.tile([128, 1024], fp32, name="pt2", tag="ps2")
                    for half in range(2):
                        for rt in range(2):
                            nc.tensor.matmul(
                                pt[:, half * 512:(half + 1) * 512],
                                uco[rt][co],
                                zd[rt][:, q * 1024 + half * 512:
                                       q * 1024 + (half + 1) * 512],
                                start=(rt == 0),
                                stop=(rt == 1),
                            )
                    # drain with bias, re-interleave d
                    dst = osb.rearrange("p (w j s) -> p w s j", j=32, s=2)[
                        :, q * 16:(q + 1) * 16]
                    src = pt.rearrange("p (w s j) -> p w s j", s=2, j=32)
                    nc.scalar.activation(dst, src, ident, bias=bia[co][:, 0:1])
                nc.sync.dma_start(out=out_d[co, :, h, :], in_=osb)

        for pool in (ps2, ps1, opool, zdpool, zpool, tpool, ypool, xpool, consts):
            pool.release()

    nc.compile()
    return nc


def _host_prep(x, U_kh, U_kw, U_kd, U_c_in, U_c_out, bias):
    """Build per-core input maps (numpy only)."""
    x = np.asarray(x)
    U_kh = np.asarray(U_kh, np.float32)
    U_kw = np.asarray(U_kw, np.float32)
    U_kd = np.asarray(U_kd, np.float32)
    U_c_in = np.asarray(U_c_in, np.float32)
    U_c_out = np.asarray(U_c_out, np.float32)
    bias = np.asarray(bias, np.float32)

    xb = np.ascontiguousarray(x[0]).astype(BF16)          # [C, S, S, S]
    xb = xb.reshape(C, S, PLANE)

    # W_k[c, r] = U_c_in[c,r] * U_kh[k,r]  -> [3, ct, rt, 128, 128]
    wkh = np.empty((3, 2, 2, 128, 128), BF16)
    for k in range(3):
        wk = (U_c_in * U_kh[k][None, :]).astype(BF16)     # [C, R]
        wkh[k] = wk.reshape(2, 128, 2, 128).transpose(0, 2, 1, 3)

    uco = U_c_out.astype(BF16).reshape(2, 128, 2, 128).transpose(0, 2, 1, 3)
    uco = np.ascontiguousarray(uco)
    ukw = np.ascontiguousarray(U_kw.T.reshape(2, 128, 3))
    ukd = np.ascontiguousarray(U_kd.T.reshape(2, 128, 3))
    bias_t = np.ascontiguousarray(bias.reshape(2, 128, 1))

    in_maps = []
    for c in range(NCORES):
        slab = np.zeros((C, HS, PLANE), BF16)
        lo, hi = c * HP - 1, c * HP + HP + 1
        s0, s1 = max(lo, 0), min(hi, S)
        slab[:, s0 - lo:HS - (hi - s1)] = xb[:, s0:s1]
        slab = np.ascontiguousarray(slab.reshape(2, 128, HS, PLANE))
        in_maps.append({
            "xs": slab, "wkh": wkh, "uco": uco, "ukw": ukw,
            "ukd": ukd, "bias_t": bias_t,
        })
    return in_maps


def kernel(x, U_kh, U_kw, U_kd, U_c_in, U_c_out, bias, _trace=False):
    from concourse.bass_utils import run_bass_kernel_spmd

    if "nc" not in _cache:
        _cache["nc"] = _build_program()
    nc = _cache["nc"]

    in_maps = _host_prep(x, U_kh, U_kw, U_kd, U_c_in, U_c_out, bias)
    res = run_bass_kernel_spmd(nc, in_maps, core_ids=list(range(NCORES)),
                               trace=_trace)
    _cache["last_result"] = res

    out = np.empty((1, CO, S, S, S), np.float32)
    for c in range(NCORES):
        o = res.results[c]["out"]                        # [2, 128, HP, PLANE] bf16
        out[0, :, c * HP:(c + 1) * HP] = o.reshape(CO, HP, S, S).astype(np.float32)
    return out



# revision 11
# speedup vs baseline: 1.2668x; 1.2668x over previous
"""Low-rank (CPD) 3D conv kernel for Trainium2, SPMD across 8 NeuronCores.

Math (per reference):
  y[r,h,w,d]  = sum_c U_c_in[c,r] * x[c,h,w,d]
  y           = conv_h(conv_w(conv_d-separable 3-tap, per-rank taps U_k*))
  out[c,...]  = sum_r U_c_out[r,c] * z[r,...] + bias[c]

Distribution: data-parallel split of H (64) into 8 slabs of 8 planes; each
core reads its slab plus one halo plane on each side (zero at global edges)
and computes its output slab independently. No collectives.

Per-core pipeline (streamed over the 8 output planes):
  - mm1 with conv_h folded: 3 weight matrices W_k = U_c_in * U_kh[k] (host
    precomputed, bf16); PSUM accumulation over 2 c-tiles x 3 h-taps.
  - PSUM drain on ScalarE, casting to bf16 and de-interleaving d into
    (even,odd) halves per w-line so the d-shifts below stay 4B-aligned.
  - conv_w on VectorE: per-partition scale (tensor_scalar) + 2 fused
    scale-add passes (scalar_tensor_tensor) with +-1 w-line shifts.
  - conv_d on VectorE: same, operating across the even/odd halves.
  - mm2: lhsT = U_c_out (bf16), accumulate 2 r-tiles.
  - PSUM drain on ScalarE with per-partition bias add, re-interleaving d,
    f32 output.
"""

import numpy as np
import ml_dtypes

BF16 = ml_dtypes.bfloat16

# Problem constants (hardcoded per contest contract)
C = 256   # input channels
R = 256   # rank
CO = 256  # output channels
S = 64    # spatial extent (cube)
NCORES = 8
HP = S // NCORES          # output planes per core (8)
HS = HP + 2               # slab planes incl. halo (10)
PLANE = S * S             # 4096 elements per (w,d) plane
NCH = PLANE // 512        # 512-column matmul chunks per plane (8)

_cache = {}


def _build_program(hp=HP, wl=S):
    """Build and compile the per-core Bass program (identical on all cores).

    hp: output planes per core; wl: w-lines per plane (64 in production).
    """
    import concourse.bass as bass
    import concourse.mybir as mybir
    import concourse.tile as tile
    from concourse import bacc

    HS, PLANE, NCH = hp + 2, wl * 64, (wl * 64) // 512
    HP_ = hp

    fp32 = mybir.dt.float32
    bf16 = mybir.dt.bfloat16

    nc = bacc.Bacc("TRN2", target_bir_lowering=False, debug=False,
                   num_devices=NCORES)

    # DRAM tensors (names are the in_map keys)
    x_d = nc.dram_tensor("xs", [2, 128, HS, PLANE], bf16, kind="ExternalInput").ap()
    wkh_d = nc.dram_tensor("wkh", [3, 2, 2, 128, 128], bf16, kind="ExternalInput").ap()
    uco_d = nc.dram_tensor("uco", [2, 2, 128, 128], bf16, kind="ExternalInput").ap()
    ukw_d = nc.dram_tensor("ukw", [2, 128, 3], fp32, kind="ExternalInput").ap()
    ukd_d = nc.dram_tensor("ukd", [2, 128, 3], fp32, kind="ExternalInput").ap()
    bias_d = nc.dram_tensor("bias_t", [2, 128, 1], fp32, kind="ExternalInput").ap()
    out_d = nc.dram_tensor("out", [2, 128, HP_, PLANE], bf16, kind="ExternalOutput").ap()

    mult = mybir.AluOpType.mult
    add = mybir.AluOpType.add
    ident = mybir.ActivationFunctionType.Identity

    with tile.TileContext(nc) as tc:
        consts = tc.alloc_tile_pool(name="consts", bufs=1)
        xpool = tc.alloc_tile_pool(name="x", bufs=8)
        ypool = tc.alloc_tile_pool(name="y", bufs=3)
        tpool = tc.alloc_tile_pool(name="tmp", bufs=5)
        zpool = tc.alloc_tile_pool(name="z", bufs=2)
        zdpool = tc.alloc_tile_pool(name="zd", bufs=4)
        opool = tc.alloc_tile_pool(name="osb", bufs=2)
        ps1 = tc.alloc_tile_pool(name="ps1", bufs=2, space="PSUM")
        ps2 = tc.alloc_tile_pool(name="ps2", bufs=2, space="PSUM")

        # ---- constants ----
        wkh = [[[consts.tile([128, 128], bf16, name=f"wkh{k}{ct}{rt}", tag=f"wkh{k}{ct}{rt}")
                 for rt in range(2)] for ct in range(2)] for k in range(3)]
        for k in range(3):
            for ct in range(2):
                for rt in range(2):
                    nc.sync.dma_start(out=wkh[k][ct][rt], in_=wkh_d[k, ct, rt])
        uco = [[consts.tile([128, 128], bf16, name=f"uco{rt}{co}", tag=f"uco{rt}{co}")
                for co in range(2)] for rt in range(2)]
        for rt in range(2):
            for co in range(2):
                nc.sync.dma_start(out=uco[rt][co], in_=uco_d[rt, co])
        ukw = [consts.tile([128, 3], fp32, name=f"ukw{rt}", tag=f"ukw{rt}") for rt in range(2)]
        ukd = [consts.tile([128, 3], fp32, name=f"ukd{rt}", tag=f"ukd{rt}") for rt in range(2)]
        bia = [consts.tile([128, 1], fp32, name=f"bias{co}", tag=f"bias{co}") for co in range(2)]
        for rt in range(2):
            nc.sync.dma_start(out=ukw[rt], in_=ukw_d[rt])
            nc.sync.dma_start(out=ukd[rt], in_=ukd_d[rt])
        for co in range(2):
            nc.sync.dma_start(out=bia[co], in_=bias_d[co])

        # ---- x plane streaming ----
        xt = {}

        def get_x(p, ct):
            if (p, ct) not in xt:
                t = xpool.tile([128, PLANE], bf16, name="xplane", tag="xplane")
                nc.sync.dma_start(out=t, in_=x_d[ct, :, p, :])
                xt[(p, ct)] = t
            return xt[(p, ct)]

        NQ = PLANE // 1024  # 1024-wide psum tiles per plane

        def mm2_stage(h, zd):
            # mm2 + bias drain (bf16 out; host upcasts)
            for co in range(2):
                osb = opool.tile([128, PLANE], bf16, name="osb", tag="osb")
                for q in range(NQ):
                    pt = ps2.tile([128, 1024], fp32, name="pt2", tag="ps2")
                    for half in range(2):
                        for rt in range(2):
                            nc.tensor.matmul(
                                pt[:, half * 512:(half + 1) * 512],
                                uco[rt][co],
                                zd[rt][:, q * 1024 + half * 512:
                                       q * 1024 + (half + 1) * 512],
                                start=(rt == 0),
                                stop=(rt == 1),
                            )
                    # drain with bias, re-interleave d
                    dst = osb.rearrange("p (w j s) -> p w s j", j=32, s=2)[
                        :, q * 16:(q + 1) * 16]
                    src = pt.rearrange("p (w s j) -> p w s j", s=2, j=32)
                    nc.scalar.activation(dst, src, ident, bias=bia[co][:, 0:1])
                nc.sync.dma_start(out=out_d[co, :, h, :], in_=osb)

        zd_prev = None
        for h in range(HP_):
            y = []
            t0s = []
            for rt in range(2):
                # --- mm1 + conv_h fold (PSUM 1024-tiles, 512 matmul halves) ---
                ysb = ypool.tile([128, PLANE], bf16, name="ysb", tag="y")
                t0 = tpool.tile([128, PLANE], bf16, name="t0t", tag="tmp")
                for q in range(NQ):
                    pt = ps1.tile([128, 1024], fp32, name="pt1", tag="ps1")
                    for half in range(2):
                        first = True
                        for k in range(3):
                            for ct in range(2):
                                nc.tensor.matmul(
                                    pt[:, half * 512:(half + 1) * 512],
                                    wkh[k][ct][rt],
                                    get_x(h + k, ct)[:, q * 1024 + half * 512:
                                                     q * 1024 + (half + 1) * 512],
                                    start=first,
                                    stop=(k == 2 and ct == 1),
                                )
                                first = False
                    # drains: f32 PSUM -> bf16 SBUF, de-interleave d.
                    # plain y (ACT); U_kw[0]-scaled t0 from ACT for rt0 and
                    # from a DVE mul for rt1 (ACT/DVE load balance).
                    src = pt.rearrange("p (w j s) -> p w s j", j=32, s=2)
                    dst = ysb.rearrange("p (w s j) -> p w s j", s=2, j=32)[
                        :, q * 16:(q + 1) * 16]
                    nc.scalar.copy(dst, src)
                    if rt == 0:
                        dst0 = t0.rearrange("p (w s j) -> p w s j", s=2, j=32)[
                            :, q * 16:(q + 1) * 16]
                        nc.scalar.mul(dst0, src, ukw[rt][:, 0:1])
                if rt == 1:
                    nc.vector.tensor_scalar_mul(t0, ysb, ukw[rt][:, 0:1])
                y.append(ysb)
                t0s.append(t0)

            # --- conv_w (VectorE + tmp from ACT drains) ---
            z = []
            for rt in range(2):
                zt = zpool.tile([128, PLANE], bf16, name="zw", tag="z")
                # z = U1*y
                nc.vector.tensor_scalar_mul(zt, y[rt], ukw[rt][:, 1:2])
                zv = zt.rearrange("p (w q) -> p w q", q=64)
                t0v = t0s[rt].rearrange("p (w q) -> p w q", q=64)
                yv = y[rt].rearrange("p (w q) -> p w q", q=64)
                # z[w] += t0[w-1]
                nc.vector.tensor_tensor(zv[:, 1:, :], t0v[:, :-1, :], zv[:, 1:, :], add)
                # t2 = U2*y ; z[w] += t2[w+1]
                t2 = tpool.tile([128, PLANE], bf16, name="t2t", tag="tmp")
                nc.vector.tensor_scalar_mul(t2, y[rt], ukw[rt][:, 2:3])
                t2v = t2.rearrange("p (w q) -> p w q", q=64)
                nc.vector.tensor_tensor(zv[:, :-1, :], t2v[:, 1:, :], zv[:, :-1, :], add)
                z.append(zt)

            # --- conv_d (VectorE scales; even-chain on DVE, odd-chain on
            # GpSimd so the two 2-op add chains run in parallel) ---
            zd = []
            for rt in range(2):
                zt = zdpool.tile([128, PLANE], bf16, name="zdt", tag="zd")
                a0 = tpool.tile([128, PLANE], bf16, name="a0t", tag="tmp")
                a2 = tpool.tile([128, PLANE], bf16, name="a2t", tag="tmp")
                nc.vector.tensor_scalar_mul(zt, z[rt], ukd[rt][:, 1:2])
                nc.vector.tensor_scalar_mul(a0, z[rt], ukd[rt][:, 0:1])
                nc.vector.tensor_scalar_mul(a2, z[rt], ukd[rt][:, 2:3])
                zv = zt.rearrange("p (w s j) -> p w s j", s=2, j=32)
                a0v = a0.rearrange("p (w s j) -> p w s j", s=2, j=32)
                a2v = a2.rearrange("p (w s j) -> p w s j", s=2, j=32)
                # 4B-aligned adds on DVE (fast 2x mode); the two j+-1 "wrap"
                # adds have 2-byte-offset APs that cripple DVE, so they go to
                # GpSimd which is alignment-insensitive.
                nc.vector.tensor_tensor(zv[:, :, 0, :], a2v[:, :, 1, :], zv[:, :, 0, :], add)
                nc.vector.tensor_tensor(zv[:, :, 1, :], a0v[:, :, 0, :], zv[:, :, 1, :], add)
                nc.gpsimd.tensor_tensor(zv[:, :, 0, 1:], a0v[:, :, 1, :-1], zv[:, :, 0, 1:], add)
                nc.gpsimd.tensor_tensor(zv[:, :, 1, :-1], a2v[:, :, 0, 1:], zv[:, :, 1, :-1], add)
                zd.append(zt)

            # software pipelining: mm2 for the PREVIOUS h, so the tensor
            # engine's program order is mm1(h), mm2(h-1), mm1(h+1), ... and
            # it never stalls waiting for the current conv chain.
            if zd_prev is not None:
                mm2_stage(h - 1, zd_prev)
            zd_prev = zd

        mm2_stage(HP_ - 1, zd_prev)

        for pool in (ps2, ps1, opool, zdpool, zpool, tpool, ypool, xpool, consts):
            pool.release()

    nc.compile()
    return nc


def _host_prep(x, U_kh, U_kw, U_kd, U_c_in, U_c_out, bias):
    """Build per-core input maps (numpy only)."""
    x = np.asarray(x)
    U_kh = np.asarray(U_kh, np.float32)
    U_kw = np.asarray(U_kw, np.float32)
    U_kd = np.asarray(U_kd, np.float32)
    U_c_in = np.asarray(U_c_in, np.float32)
    U_c_out = np.asarray(U_c_out, np.float32)
    bias = np.asarray(bias, np.float32)

    xb = np.ascontiguousarray(x[0]).astype(BF16)          # [C, S, S, S]
    xb = xb.reshape(C, S, PLANE)

    # W_k[c, r] = U_c_in[c,r] * U_kh[k,r]  -> [3, ct, rt, 128, 128]
    wkh = np.empty((3, 2, 2, 128, 128), BF16)
    for k in range(3):
        wk = (U_c_in * U_kh[k][None, :]).astype(BF16)     # [C, R]
        wkh[k] = wk.reshape(2, 128, 2, 128).transpose(0, 2, 1, 3)

    uco = U_c_out.astype(BF16).reshape(2, 128, 2, 128).transpose(0, 2, 1, 3)
    uco = np.ascontiguousarray(uco)
    ukw = np.ascontiguousarray(U_kw.T.reshape(2, 128, 3))
    ukd = np.ascontiguousarray(U_kd.T.reshape(2, 128, 3))
    bias_t = np.ascontiguousarray(bias.reshape(2, 128, 1))

    in_maps = []
    for c in range(NCORES):
        slab = np.zeros((C, HS, PLANE), BF16)
        lo, hi = c * HP - 1, c * HP + HP + 1
        s0, s1 = max(lo, 0), min(hi, S)
        slab[:, s0 - lo:HS - (hi - s1)] = xb[:, s0:s1]
        slab = np.ascontiguousarray(slab.reshape(2, 128, HS, PLANE))
        in_maps.append({
            "xs": slab, "wkh": wkh, "uco": uco, "ukw": ukw,
            "ukd": ukd, "bias_t": bias_t,
        })
    return in_maps


def kernel(x, U_kh, U_kw, U_kd, U_c_in, U_c_out, bias, _trace=False):
    from concourse.bass_utils import run_bass_kernel_spmd

    if "nc" not in _cache:
        _cache["nc"] = _build_program()
    nc = _cache["nc"]

    in_maps = _host_prep(x, U_kh, U_kw, U_kd, U_c_in, U_c_out, bias)
    res = run_bass_kernel_spmd(nc, in_maps, core_ids=list(range(NCORES)),
                               trace=_trace)
    _cache["last_result"] = res

    out = np.empty((1, CO, S, S, S), np.float32)
    for c in range(NCORES):
        o = res.results[c]["out"]                        # [2, 128, HP, PLANE] bf16
        out[0, :, c * HP:(c + 1) * HP] = o.reshape(CO, HP, S, S).astype(np.float32)
    return out



# revision 12
# speedup vs baseline: 1.3850x; 1.0933x over previous
"""Low-rank (CPD) 3D conv kernel for Trainium2, SPMD across 8 NeuronCores.

Math (per reference):
  y[r,h,w,d]  = sum_c U_c_in[c,r] * x[c,h,w,d]
  z           = conv_h/w/d separable 3-tap convs with per-rank taps U_k*
  out[c,...]  = sum_r U_c_out[r,c] * z[r,...] + bias[c]

Distribution: data-parallel split of H (64) into 8 slabs of 8 planes; each
core reads its slab plus one halo plane on each side (zero at global edges)
and computes its output slab independently. No collectives.

Per-core pipeline, software-pipelined over the 8 output planes (mm2 for
plane h is emitted during iteration h+1 so the tensor engine never stalls
on the conv chain):
  - mm1 with conv_h folded: 3 weight matrices W_k = U_c_in * U_kh[k] (host
    precomputed, bf16); PSUM accumulation over 2 c-tiles x 3 h-taps.
  - ACT drains PSUM twice, casting to bf16: zc = U_kw[1]*y and t0 =
    U_kw[0]*y (the conv_w center + left-tap scalings ride the drain for
    free).
  - conv_w on DVE: t2 = (U_kw[2]/U_kw[1])*zc, then two in-place shifted
    adds (+-1 w-line = +-64 elements, 4B-aligned so DVE runs in fast mode).
  - conv_d: 3 DVE scale muls; the +-1-element d-shifts are 2-byte-misaligned
    (cripples DVE) so they are done as flat DMA shifted copies (1
    descriptor/partition, issued from the GpSimd sequencer - no engine
    contention) + DVE memset of the contaminated d-edge columns + two
    aligned full-plane DVE adds.
  - mm2: lhsT = U_c_out (bf16), accumulate 2 r-tiles; ACT drain adds bias
    and writes bf16 (host upcasts to f32).

GpSimd ALU ops are avoided entirely: concurrent GpSimd execution disables
DVE's 2-port fast modes (5-7x slowdown measured).
"""

import numpy as np
import ml_dtypes

BF16 = ml_dtypes.bfloat16

# Problem constants (hardcoded per contest contract)
C = 256   # input channels
R = 256   # rank
CO = 256  # output channels
S = 64    # spatial extent (cube)
NCORES = 8
HP = S // NCORES          # output planes per core (8)
HS = HP + 2               # slab planes incl. halo (10)
PLANE = S * S             # 4096 elements per (w,d) plane

_cache = {}


def _build_program(hp=HP):
    import concourse.bass as bass
    import concourse.mybir as mybir
    import concourse.tile as tile
    from concourse import bacc

    HS_, HP_ = hp + 2, hp

    fp32 = mybir.dt.float32
    bf16 = mybir.dt.bfloat16

    nc = bacc.Bacc("TRN2", target_bir_lowering=False, debug=False,
                   num_devices=NCORES)

    x_d = nc.dram_tensor("xs", [2, 128, HS_, PLANE], bf16, kind="ExternalInput").ap()
    wkh_d = nc.dram_tensor("wkh", [3, 2, 2, 128, 128], bf16, kind="ExternalInput").ap()
    uco_d = nc.dram_tensor("uco", [2, 2, 128, 128], bf16, kind="ExternalInput").ap()
    ukw_d = nc.dram_tensor("ukw", [2, 128, 3], fp32, kind="ExternalInput").ap()
    ukd_d = nc.dram_tensor("ukd", [2, 128, 3], fp32, kind="ExternalInput").ap()
    bias_d = nc.dram_tensor("bias_t", [2, 128, 1], fp32, kind="ExternalInput").ap()
    out_d = nc.dram_tensor("out", [2, 128, HP_, PLANE], bf16, kind="ExternalOutput").ap()

    mult = mybir.AluOpType.mult
    add = mybir.AluOpType.add
    ident = mybir.ActivationFunctionType.Identity

    with tile.TileContext(nc) as tc:
        consts = tc.alloc_tile_pool(name="consts", bufs=1)
        xpool = tc.alloc_tile_pool(name="x", bufs=8)
        zcpool = tc.alloc_tile_pool(name="zc", bufs=3)
        tpool = tc.alloc_tile_pool(name="tmp", bufs=7)
        zdpool = tc.alloc_tile_pool(name="zd", bufs=4)
        opool = tc.alloc_tile_pool(name="osb", bufs=2)
        ps1 = tc.alloc_tile_pool(name="ps1", bufs=2, space="PSUM")
        ps2 = tc.alloc_tile_pool(name="ps2", bufs=2, space="PSUM")

        # ---- constants ----
        wkh = [[[consts.tile([128, 128], bf16, name=f"wkh{k}{ct}{rt}", tag=f"wkh{k}{ct}{rt}")
                 for rt in range(2)] for ct in range(2)] for k in range(3)]
        for k in range(3):
            for ct in range(2):
                for rt in range(2):
                    nc.sync.dma_start(out=wkh[k][ct][rt], in_=wkh_d[k, ct, rt])
        uco = [[consts.tile([128, 128], bf16, name=f"uco{rt}{co}", tag=f"uco{rt}{co}")
                for co in range(2)] for rt in range(2)]
        for rt in range(2):
            for co in range(2):
                nc.sync.dma_start(out=uco[rt][co], in_=uco_d[rt, co])
        # ukw columns: [Uw0, Uw1, Uw2/Uw1(clamped)]; ukd: raw taps
        ukw = [consts.tile([128, 3], fp32, name=f"ukw{rt}", tag=f"ukw{rt}") for rt in range(2)]
        ukd = [consts.tile([128, 3], fp32, name=f"ukd{rt}", tag=f"ukd{rt}") for rt in range(2)]
        bia = [consts.tile([128, 1], fp32, name=f"bias{co}", tag=f"bias{co}") for co in range(2)]
        for rt in range(2):
            nc.sync.dma_start(out=ukw[rt], in_=ukw_d[rt])
            nc.sync.dma_start(out=ukd[rt], in_=ukd_d[rt])
        for co in range(2):
            nc.sync.dma_start(out=bia[co], in_=bias_d[co])

        # ---- x plane streaming ----
        xt = {}

        def get_x(p, ct):
            if (p, ct) not in xt:
                t = xpool.tile([128, PLANE], bf16, name="xplane", tag="xplane")
                nc.sync.dma_start(out=t, in_=x_d[ct, :, p, :])
                xt[(p, ct)] = t
            return xt[(p, ct)]

        NQ = PLANE // 1024  # 1024-wide psum tiles per plane

        def mm2_stage(h, zd):
            # mm2 + bias drain (bf16 out; host upcasts)
            for co in range(2):
                osb = opool.tile([128, PLANE], bf16, name="osb", tag="osb")
                for q in range(NQ):
                    pt = ps2.tile([128, 1024], fp32, name="pt2", tag="ps2")
                    for half in range(2):
                        for rt in range(2):
                            nc.tensor.matmul(
                                pt[:, half * 512:(half + 1) * 512],
                                uco[rt][co],
                                zd[rt][:, q * 1024 + half * 512:
                                       q * 1024 + (half + 1) * 512],
                                start=(rt == 0),
                                stop=(rt == 1),
                            )
                    nc.scalar.activation(osb[:, q * 1024:(q + 1) * 1024], pt,
                                         ident, bias=bia[co][:, 0:1])
                nc.sync.dma_start(out=out_d[co, :, h, :], in_=osb)

        zd_prev = None
        for h in range(HP_):
            zd = []
            for rt in range(2):
                # --- mm1 + conv_h fold ---
                zc = zcpool.tile([128, PLANE], bf16, name="zc", tag="zc")
                t0 = tpool.tile([128, PLANE], bf16, name="t0t", tag="tmp")
                for q in range(NQ):
                    pt = ps1.tile([128, 1024], fp32, name="pt1", tag="ps1")
                    for half in range(2):
                        first = True
                        for k in range(3):
                            for ct in range(2):
                                nc.tensor.matmul(
                                    pt[:, half * 512:(half + 1) * 512],
                                    wkh[k][ct][rt],
                                    get_x(h + k, ct)[:, q * 1024 + half * 512:
                                                     q * 1024 + (half + 1) * 512],
                                    start=first,
                                    stop=(k == 2 and ct == 1),
                                )
                                first = False
                    # ACT drains: zc = Uw1*y, t0 = Uw0*y (f32 PSUM -> bf16)
                    dst = slice(q * 1024, (q + 1) * 1024)
                    nc.scalar.mul(zc[:, dst], pt, ukw[rt][:, 1:2])
                    nc.scalar.mul(t0[:, dst], pt, ukw[rt][:, 0:1])

                # --- conv_w (DVE; +-64-element shifts, all aligned) ---
                t2 = tpool.tile([128, PLANE], bf16, name="t2t", tag="tmp")
                nc.vector.tensor_scalar_mul(t2, zc, ukw[rt][:, 2:3])
                # in-place: zc becomes z
                nc.vector.tensor_tensor(zc[:, 64:], t0[:, :PLANE - 64], zc[:, 64:], add)
                nc.vector.tensor_tensor(zc[:, :PLANE - 64], t2[:, 64:], zc[:, :PLANE - 64], add)

                # --- conv_d ---
                zt = zdpool.tile([128, PLANE], bf16, name="zdt", tag="zd")
                a0 = tpool.tile([128, PLANE], bf16, name="a0t", tag="tmp")
                a2 = tpool.tile([128, PLANE], bf16, name="a2t", tag="tmp")
                nc.vector.tensor_scalar_mul(zt, zc, ukd[rt][:, 1:2])
                nc.vector.tensor_scalar_mul(a0, zc, ukd[rt][:, 0:1])
                nc.vector.tensor_scalar_mul(a2, zc, ukd[rt][:, 2:3])
                # flat +-1-element shifts via DMA (no DVE misalignment penalty)
                a0s = tpool.tile([128, PLANE], bf16, name="a0s", tag="tmp")
                a2s = tpool.tile([128, PLANE], bf16, name="a2s", tag="tmp")
                nc.gpsimd.dma_start(out=a0s[:, 1:], in_=a0[:, :PLANE - 1])
                nc.gpsimd.dma_start(out=a2s[:, :PLANE - 1], in_=a2[:, 1:])
                # zero the cross-w-line contaminated columns (d=0 / d=63)
                a0v = a0s.rearrange("p (w d) -> p w d", d=64)
                a2v = a2s.rearrange("p (w d) -> p w d", d=64)
                nc.vector.memset(a0v[:, :, 0:1], 0.0)
                nc.vector.memset(a2v[:, :, 63:64], 0.0)
                nc.vector.tensor_tensor(zt, a0s, zt, add)
                nc.vector.tensor_tensor(zt, a2s, zt, add)
                zd.append(zt)

            # software pipelining: emit mm2 for the PREVIOUS h so the tensor
            # engine's program order is mm1(h), mm2(h-1), mm1(h+1), ...
            if zd_prev is not None:
                mm2_stage(h - 1, zd_prev)
            zd_prev = zd

        mm2_stage(HP_ - 1, zd_prev)

        for pool in (ps2, ps1, opool, zdpool, tpool, zcpool, xpool, consts):
            pool.release()

    nc.compile()
    return nc


def _host_prep(x, U_kh, U_kw, U_kd, U_c_in, U_c_out, bias):
    """Build per-core input maps (numpy only)."""
    x = np.asarray(x)
    U_kh = np.asarray(U_kh, np.float32)
    U_kw = np.asarray(U_kw, np.float32)
    U_kd = np.asarray(U_kd, np.float32)
    U_c_in = np.asarray(U_c_in, np.float32)
    U_c_out = np.asarray(U_c_out, np.float32)
    bias = np.asarray(bias, np.float32)

    xb = np.ascontiguousarray(x[0]).astype(BF16)          # [C, S, S, S]
    xb = xb.reshape(C, S, PLANE)

    # W_k[c, r] = U_c_in[c,r] * U_kh[k,r]  -> [3, ct, rt, 128, 128]
    wkh = np.empty((3, 2, 2, 128, 128), BF16)
    for k in range(3):
        wk = (U_c_in * U_kh[k][None, :]).astype(BF16)     # [C, R]
        wkh[k] = wk.reshape(2, 128, 2, 128).transpose(0, 2, 1, 3)

    uco = U_c_out.astype(BF16).reshape(2, 128, 2, 128).transpose(0, 2, 1, 3)
    uco = np.ascontiguousarray(uco)

    # conv_w scalars: [Uw0, Uw1, Uw2/Uw1] (Uw1 clamped away from 0)
    uw = U_kw.T.copy()                                    # [R, 3]
    uw1 = uw[:, 1].copy()
    tiny = np.float32(1e-30)
    uw1[np.abs(uw1) < tiny] = tiny
    uw[:, 1] = uw1
    uw[:, 2] = uw[:, 2] / uw1
    ukw = np.ascontiguousarray(uw.reshape(2, 128, 3).astype(np.float32))
    ukd = np.ascontiguousarray(U_kd.T.reshape(2, 128, 3))
    bias_t = np.ascontiguousarray(bias.reshape(2, 128, 1))

    in_maps = []
    for c in range(NCORES):
        slab = np.zeros((C, HS, PLANE), BF16)
        lo, hi = c * HP - 1, c * HP + HP + 1
        s0, s1 = max(lo, 0), min(hi, S)
        slab[:, s0 - lo:HS - (hi - s1)] = xb[:, s0:s1]
        slab = np.ascontiguousarray(slab.reshape(2, 128, HS, PLANE))
        in_maps.append({
            "xs": slab, "wkh": wkh, "uco": uco, "ukw": ukw,
            "ukd": ukd, "bias_t": bias_t,
        })
    return in_maps


def kernel(x, U_kh, U_kw, U_kd, U_c_in, U_c_out, bias, _trace=False):
    from concourse.bass_utils import run_bass_kernel_spmd

    if "nc" not in _cache:
        _cache["nc"] = _build_program()
    nc = _cache["nc"]

    in_maps = _host_prep(x, U_kh, U_kw, U_kd, U_c_in, U_c_out, bias)
    res = run_bass_kernel_spmd(nc, in_maps, core_ids=list(range(NCORES)),
                               trace=_trace)
    _cache["last_result"] = res

    out = np.empty((1, CO, S, S, S), np.float32)
    for c in range(NCORES):
        o = res.results[c]["out"]                        # [2, 128, HP, PLANE] bf16
        out[0, :, c * HP:(c + 1) * HP] = o.reshape(CO, HP, S, S).astype(np.float32)
    return out


# revision 15
# speedup vs baseline: 1.4826x; 1.0705x over previous
"""Low-rank (CPD) 3D conv kernel for Trainium2, SPMD across 8 NeuronCores.

Math (per reference):
  y[r,h,w,d]  = sum_c U_c_in[c,r] * x[c,h,w,d]
  z           = conv_h/w/d separable 3-tap convs with per-rank taps U_k*
  out[c,...]  = sum_r U_c_out[r,c] * z[r,...] + bias[c]

Distribution: data-parallel split of H (64) into 8 slabs of 8 planes; each
core reads its slab plus one halo plane on each side (zero at global edges)
and computes its output slab independently. No collectives.

Per-core pipeline, software-pipelined over the 8 output planes (mm2 for
plane h is emitted during iteration h+1 so the tensor engine never stalls
on the conv chain):
  - mm1 with conv_h folded: 3 weight matrices W_k = U_c_in * U_kh[k] (host
    precomputed, bf16); PSUM accumulation over 2 c-tiles x 3 h-taps.
  - ACT drains PSUM twice, casting to bf16: zc = U_kw[1]*y and t0 =
    U_kw[0]*y (the conv_w center + left-tap scalings ride the drain for
    free).
  - conv_w on DVE: t2 = (U_kw[2]/U_kw[1])*zc, then two in-place shifted
    adds (+-1 w-line = +-64 elements, 4B-aligned so DVE runs in fast mode).
  - conv_d: 3 DVE scale muls; the +-1-element d-shifts are 2-byte-misaligned
    (cripples DVE) so they are done as flat DMA shifted copies (1
    descriptor/partition, issued from the GpSimd sequencer - no engine
    contention) + DVE memset of the contaminated d-edge columns + two
    aligned full-plane DVE adds.
  - mm2: lhsT = U_c_out (bf16), accumulate 2 r-tiles; ACT drain adds bias
    and writes bf16 (host upcasts to f32).

GpSimd ALU ops are avoided entirely: concurrent GpSimd execution disables
DVE's 2-port fast modes (5-7x slowdown measured).
"""

import numpy as np
import ml_dtypes

BF16 = ml_dtypes.bfloat16

# Problem constants (hardcoded per contest contract)
C = 256   # input channels
R = 256   # rank
CO = 256  # output channels
S = 64    # spatial extent (cube)
NCORES = 8
HP = S // NCORES          # output planes per core (8)
HS = HP + 2               # slab planes incl. halo (10)
PLANE = S * S             # 4096 elements per (w,d) plane

_cache = {}


def _build_program(hp=HP):
    import concourse.bass as bass
    import concourse.mybir as mybir
    import concourse.tile as tile
    from concourse import bacc

    HS_, HP_ = hp + 2, hp

    fp32 = mybir.dt.float32
    bf16 = mybir.dt.bfloat16

    nc = bacc.Bacc("TRN2", target_bir_lowering=False, debug=False,
                   num_devices=NCORES)

    x_d = nc.dram_tensor("xs", [2, 128, HS_, PLANE], bf16, kind="ExternalInput").ap()
    wkh_d = nc.dram_tensor("wkh", [3, 2, 2, 128, 128], bf16, kind="ExternalInput").ap()
    uco_d = nc.dram_tensor("uco", [2, 2, 128, 128], bf16, kind="ExternalInput").ap()
    ukw_d = nc.dram_tensor("ukw", [2, 128, 3], fp32, kind="ExternalInput").ap()
    ukd_d = nc.dram_tensor("ukd", [2, 128, 3], fp32, kind="ExternalInput").ap()
    bias_d = nc.dram_tensor("bias_t", [2, 128, 1], fp32, kind="ExternalInput").ap()
    out_d = nc.dram_tensor("out", [2, 128, HP_, PLANE], bf16, kind="ExternalOutput").ap()

    mult = mybir.AluOpType.mult
    add = mybir.AluOpType.add
    ident = mybir.ActivationFunctionType.Identity

    with tile.TileContext(nc) as tc:
        consts = tc.alloc_tile_pool(name="consts", bufs=1)
        xpool = tc.alloc_tile_pool(name="x", bufs=8)
        zcpool = tc.alloc_tile_pool(name="zc", bufs=3)
        tpool = tc.alloc_tile_pool(name="tmp", bufs=6)
        zdpool = tc.alloc_tile_pool(name="zd", bufs=5)
        opool = tc.alloc_tile_pool(name="osb", bufs=2)
        ps1 = tc.alloc_tile_pool(name="ps1", bufs=2, space="PSUM")
        ps2 = tc.alloc_tile_pool(name="ps2", bufs=2, space="PSUM")

        # ---- constants ----
        wkh = [[[consts.tile([128, 128], bf16, name=f"wkh{k}{ct}{rt}", tag=f"wkh{k}{ct}{rt}")
                 for rt in range(2)] for ct in range(2)] for k in range(3)]
        for k in range(3):
            for ct in range(2):
                for rt in range(2):
                    nc.sync.dma_start(out=wkh[k][ct][rt], in_=wkh_d[k, ct, rt])
        uco = [[consts.tile([128, 128], bf16, name=f"uco{rt}{co}", tag=f"uco{rt}{co}")
                for co in range(2)] for rt in range(2)]
        for rt in range(2):
            for co in range(2):
                nc.sync.dma_start(out=uco[rt][co], in_=uco_d[rt, co])
        # ukw columns: [Uw0, Uw1, Uw2/Uw1(clamped)]; ukd: raw taps
        ukw = [consts.tile([128, 3], fp32, name=f"ukw{rt}", tag=f"ukw{rt}") for rt in range(2)]
        ukd = [consts.tile([128, 3], fp32, name=f"ukd{rt}", tag=f"ukd{rt}") for rt in range(2)]
        bia = [consts.tile([128, 1], fp32, name=f"bias{co}", tag=f"bias{co}") for co in range(2)]
        for rt in range(2):
            nc.sync.dma_start(out=ukw[rt], in_=ukw_d[rt])
            nc.sync.dma_start(out=ukd[rt], in_=ukd_d[rt])
        for co in range(2):
            nc.sync.dma_start(out=bia[co], in_=bias_d[co])

        # ---- x plane streaming ----
        xt = {}

        def get_x(p, ct):
            if (p, ct) not in xt:
                t = xpool.tile([128, PLANE], bf16, name="xplane", tag="xplane")
                nc.sync.dma_start(out=t, in_=x_d[ct, :, p, :])
                xt[(p, ct)] = t
            return xt[(p, ct)]

        NQ = PLANE // 1024  # 1024-wide psum tiles per plane

        def mm2_stage(h, zd):
            # mm2 + bias drain (bf16 out; host upcasts)
            for co in range(2):
                osb = opool.tile([128, PLANE], bf16, name="osb", tag="osb")
                for q in range(NQ):
                    pt = ps2.tile([128, 1024], fp32, name="pt2", tag="ps2")
                    for half in range(2):
                        for rt in range(2):
                            nc.tensor.matmul(
                                pt[:, half * 512:(half + 1) * 512],
                                uco[rt][co],
                                zd[rt][:, q * 1024 + half * 512:
                                       q * 1024 + (half + 1) * 512],
                                start=(rt == 0),
                                stop=(rt == 1),
                            )
                    nc.scalar.activation(osb[:, q * 1024:(q + 1) * 1024], pt,
                                         ident, bias=bia[co][:, 0:1])
                nc.sync.dma_start(out=out_d[co, :, h, :], in_=osb)

        zd_prev = None
        for h in range(HP_):
            zd = []
            for rt in range(2):
                # --- mm1 + conv_h fold ---
                zc = zcpool.tile([128, PLANE], bf16, name="zc", tag="zc")
                t0 = tpool.tile([128, PLANE], bf16, name="t0t", tag="tmp")
                for q in range(NQ):
                    pt = ps1.tile([128, 1024], fp32, name="pt1", tag="ps1")
                    for half in range(2):
                        first = True
                        for k in range(3):
                            for ct in range(2):
                                nc.tensor.matmul(
                                    pt[:, half * 512:(half + 1) * 512],
                                    wkh[k][ct][rt],
                                    get_x(h + k, ct)[:, q * 1024 + half * 512:
                                                     q * 1024 + (half + 1) * 512],
                                    start=first,
                                    stop=(k == 2 and ct == 1),
                                )
                                first = False
                    # ACT drain: zc = Uw1*y (f32 PSUM -> bf16)
                    dst = slice(q * 1024, (q + 1) * 1024)
                    nc.scalar.mul(zc[:, dst], pt, ukw[rt][:, 1:2])

                # --- conv_w (DVE; +-64-element shifts, all aligned) ---
                # t0 = (Uw0/Uw1)*zc, t2 = (Uw2/Uw1)*zc (host pre-divided)
                t2 = tpool.tile([128, PLANE], bf16, name="t2t", tag="tmp")
                nc.vector.tensor_scalar_mul(t0, zc, ukw[rt][:, 0:1])
                nc.vector.tensor_scalar_mul(t2, zc, ukw[rt][:, 2:3])
                # in-place: zc becomes z
                nc.vector.tensor_tensor(zc[:, 64:], t0[:, :PLANE - 64], zc[:, 64:], add)
                nc.vector.tensor_tensor(zc[:, :PLANE - 64], t2[:, 64:], zc[:, :PLANE - 64], add)

                # --- conv_d ---
                zt = zdpool.tile([128, PLANE], bf16, name="zdt", tag="zd")
                a0 = tpool.tile([128, PLANE], bf16, name="a0t", tag="tmp")
                a2 = tpool.tile([128, PLANE], bf16, name="a2t", tag="tmp")
                nc.vector.tensor_scalar_mul(zt, zc, ukd[rt][:, 1:2])
                nc.vector.tensor_scalar_mul(a0, zc, ukd[rt][:, 0:1])
                nc.vector.tensor_scalar_mul(a2, zc, ukd[rt][:, 2:3])
                # flat +-1-element shifts via DMA (no DVE misalignment penalty)
                a0s = tpool.tile([128, PLANE], bf16, name="a0s", tag="tmp")
                a2s = tpool.tile([128, PLANE], bf16, name="a2s", tag="tmp")
                nc.gpsimd.dma_start(out=a0s[:, 1:], in_=a0[:, :PLANE - 1])
                nc.gpsimd.dma_start(out=a2s[:, :PLANE - 1], in_=a2[:, 1:])
                # zero the cross-w-line contaminated columns (d=0 / d=63)
                a0v = a0s.rearrange("p (w d) -> p w d", d=64)
                a2v = a2s.rearrange("p (w d) -> p w d", d=64)
                nc.vector.memset(a0v[:, :, 0:1], 0.0)
                nc.vector.memset(a2v[:, :, 63:64], 0.0)
                nc.vector.tensor_tensor(zt, a0s, zt, add)
                nc.vector.tensor_tensor(zt, a2s, zt, add)
                zd.append(zt)

            # software pipelining: emit mm2 for the PREVIOUS h so the tensor
            # engine's program order is mm1(h), mm2(h-1), mm1(h+1), ...
            if zd_prev is not None:
                mm2_stage(h - 1, zd_prev)
            zd_prev = zd

        mm2_stage(HP_ - 1, zd_prev)

        for pool in (ps2, ps1, opool, zdpool, tpool, zcpool, xpool, consts):
            pool.release()

    nc.compile()
    return nc


def _host_prep(x, U_kh, U_kw, U_kd, U_c_in, U_c_out, bias):
    """Build per-core input maps (numpy only)."""
    x = np.asarray(x)
    U_kh = np.asarray(U_kh, np.float32)
    U_kw = np.asarray(U_kw, np.float32)
    U_kd = np.asarray(U_kd, np.float32)
    U_c_in = np.asarray(U_c_in, np.float32)
    U_c_out = np.asarray(U_c_out, np.float32)
    bias = np.asarray(bias, np.float32)

    xb = np.ascontiguousarray(x[0]).astype(BF16)          # [C, S, S, S]
    xb = xb.reshape(C, S, PLANE)

    # W_k[c, r] = U_c_in[c,r] * U_kh[k,r]  -> [3, ct, rt, 128, 128]
    wkh = np.empty((3, 2, 2, 128, 128), BF16)
    for k in range(3):
        wk = (U_c_in * U_kh[k][None, :]).astype(BF16)     # [C, R]
        wkh[k] = wk.reshape(2, 128, 2, 128).transpose(0, 2, 1, 3)

    uco = U_c_out.astype(BF16).reshape(2, 128, 2, 128).transpose(0, 2, 1, 3)
    uco = np.ascontiguousarray(uco)

    # conv_w scalars: [Uw0/Uw1, Uw1, Uw2/Uw1] (Uw1 clamped away from 0)
    uw = U_kw.T.copy()                                    # [R, 3]
    uw1 = uw[:, 1].copy()
    tiny = np.float32(1e-30)
    uw1[np.abs(uw1) < tiny] = tiny
    uw[:, 1] = uw1
    uw[:, 0] = uw[:, 0] / uw1
    uw[:, 2] = uw[:, 2] / uw1
    ukw = np.ascontiguousarray(uw.reshape(2, 128, 3).astype(np.float32))
    ukd = np.ascontiguousarray(U_kd.T.reshape(2, 128, 3))
    bias_t = np.ascontiguousarray(bias.reshape(2, 128, 1))

    in_maps = []
    for c in range(NCORES):
        slab = np.zeros((C, HS, PLANE), BF16)
        lo, hi = c * HP - 1, c * HP + HP + 1
        s0, s1 = max(lo, 0), min(hi, S)
        slab[:, s0 - lo:HS - (hi - s1)] = xb[:, s0:s1]
        slab = np.ascontiguousarray(slab.reshape(2, 128, HS, PLANE))
        in_maps.append({
            "xs": slab, "wkh": wkh, "uco": uco, "ukw": ukw,
            "ukd": ukd, "bias_t": bias_t,
        })
    return in_maps


def kernel(x, U_kh, U_kw, U_kd, U_c_in, U_c_out, bias, _trace=False):
    from concourse.bass_utils import run_bass_kernel_spmd

    if "nc" not in _cache:
        _cache["nc"] = _build_program()
    nc = _cache["nc"]

    in_maps = _host_prep(x, U_kh, U_kw, U_kd, U_c_in, U_c_out, bias)
    res = run_bass_kernel_spmd(nc, in_maps, core_ids=list(range(NCORES)),
                               trace=_trace)
    _cache["last_result"] = res

    out = np.empty((1, CO, S, S, S), np.float32)
    for c in range(NCORES):
        o = res.results[c]["out"]                        # [2, 128, HP, PLANE] bf16
        out[0, :, c * HP:(c + 1) * HP] = o.reshape(CO, HP, S, S).astype(np.float32)
    return out
